# revision 16
# baseline (speedup 1.0000x reference)
"""Causal self-attention on 8 Trainium2 NeuronCores.

Sharding: B*H = 2*12 = 24 (batch, head) pairs -> 3 heads per core.
Core i handles batch i//4, heads 3*(i%4) .. 3*(i%4)+2.

v2 design (vs baseline):
  - Host passes xT = x.T [768, T] so no PE transposes / DVE copies are
    needed to feed the projections (t-on-free layout comes straight from
    DRAM with 2KB contiguous lines).
  - Projections and attention are fused in one loop over 512-row blocks
    (tb): project block tb, then run attention for q-block tb (whose
    causal k-range ends at block tb).  ACT exp work overlaps PE
    projection work throughout instead of phase-serializing.
  - q-blocks are 512 wide: every attention matmul moves 512 columns
    (the fp32 max), halving instruction count vs 256-wide blocks.
  - Causal masking via gpsimd affine_select on the otherwise-idle Pool
    engine (computed predicate, no mask tile, off the DVE).
  - Epilogue reads acc directly from PSUM (reciprocal + broadcast
    matmul + normalize), no staging copy.

Per-head layout (partition-base alignment for matmul operands):
  q01 [128,512] = qT_h0 (rows 0:64) | qT_h1 (rows 64:128)   per tb
  k01 [128,512] = kT_h0 | kT_h1                              per tb
  qv0 [128,512] = qT_h2 | vT_h0
  kv1 [128,512] = kT_h2 | vT_h1
  v2t [64,512]  = vT_h2
k01/kv1 persist across tbs (k history); q/v tiles rotate.  v is
PE-transposed to natural [t, 64] per 128-chunk with a ones column
appended at col 64 (for softmax denominators): vss[h][tb] [128, 4*65].
No max-subtraction in softmax: logits here have |.| <~ 2, exp is safe.
"""

import ml_dtypes
import numpy as np

import concourse.bass as bass
import concourse.mybir as mybir
from concourse import bacc
from concourse import tile
from concourse.bass_utils import run_bass_kernel_spmd
from concourse.masks import make_identity

F32 = mybir.dt.float32
F32R = mybir.dt.float32r
BF16 = mybir.dt.bfloat16

EMBED = 768
NHEAD = 12
DH = 64
B = 2
T = 4096
HPC = 3          # heads per core
CH = HPC * DH    # 192 channels per core
NCORES = 8


def build_program(t=T):
    """Build the single-core SPMD Bass program."""
    ntb = t // 512   # fused projection/attention blocks of 512 rows

    nc = bacc.Bacc("TRN2", target_bir_lowering=False, debug=False,
                   num_devices=NCORES)

    xT_d = nc.dram_tensor("xT", [EMBED, t], BF16, kind="ExternalInput")
    # columns: q0,q1 | k0,k1 | q2,v0 | k2,v1 | v2   (64 each)
    wqkv_d = nc.dram_tensor("wqkvT", [EMBED, 576], BF16, kind="ExternalInput")
    bqkv_d = nc.dram_tensor("bqkv", [576, 1], F32, kind="ExternalInput")
    wo_d = nc.dram_tensor("woT", [CH, EMBED], BF16, kind="ExternalInput")
    y_d = nc.dram_tensor("y", [t, EMBED], BF16, kind="ExternalOutput")

    Act = mybir.ActivationFunctionType

    with tile.TileContext(nc) as tc:
        with (
            tc.tile_pool(name="const", bufs=1) as cpool,
            tc.tile_pool(name="persist", bufs=1) as perm,
        ):
            ident = cpool.tile([128, 128], F32, tag="ident")
            make_identity(nc, ident)
            # all-ones row at partition 64 (for denominator broadcast mm)
            ones65 = cpool.tile([65, 64], F32R, tag="ones65")
            nc.gpsimd.memset(ones65.bitcast(F32), 1.0)

            # weights (DMA straight into fp32r tiles via bitcast views)
            wqkv_sb = []
            for kt in range(6):
                w_t = cpool.tile([128, 576], BF16, name=f"wqkv{kt}",
                                 tag=f"wqkv{kt}")
                nc.sync.dma_start(w_t,
                                  wqkv_d[kt * 128:(kt + 1) * 128, :])
                wqkv_sb.append(w_t)
            bias_sb = {}
            for mc in (0, 2):
                b_t = cpool.tile([128, 1], F32, name=f"bias{mc}",
                                 tag=f"bias{mc}")
                nc.sync.dma_start(b_t,
                                  bqkv_d[mc * 128:mc * 128 + 128, :])
                bias_sb[mc] = b_t
            wo_sb = []
            for h in range(3):
                wo_h = cpool.tile([64, EMBED], BF16, name=f"wo{h}",
                                  tag=f"wo{h}")
                nc.sync.dma_start(wo_h, wo_d[h * 64:(h + 1) * 64, :])
                wo_sb.append(wo_h)

            # persistent k history + natural v (65-wide chunks, col 64 = 1)
            k01s = [perm.tile([128, 512], F32R, tag=f"k01_{tb}",
                              name=f"k01_{tb}")
                    for tb in range(ntb)]
            kv1s = [perm.tile([128, 512], F32R, tag=f"kv1_{tb}",
                              name=f"kv1_{tb}")
                    for tb in range(ntb)]
            vss = [[perm.tile([128, 4 * 65], BF16, tag=f"vs{h}_{tb}",
                              name=f"vs{h}_{tb}")
                    for tb in range(ntb)] for h in range(3)]
            for h in range(3):
                for tb in range(ntb):
                    nc.gpsimd.memset(vss[h][tb], 1.0)

            with (
                tc.tile_pool(name="xt", bufs=3) as xtp,
                tc.tile_pool(name="qround", bufs=3) as qrp,
                tc.tile_pool(name="projps", bufs=2, space="PSUM") as projps,
                tc.tile_pool(name="spps", bufs=2, space="PSUM") as spps,
                tc.tile_pool(name="accps", bufs=1, space="PSUM") as accps,
                tc.tile_pool(name="ypps", bufs=1, space="PSUM") as ypps,
                tc.tile_pool(name="ptp", bufs=4) as ptp,
                tc.tile_pool(name="attnp", bufs=8) as attnp,
                tc.tile_pool(name="recp", bufs=3) as recp,
                tc.tile_pool(name="ysp", bufs=4) as ysp,
            ):
                def dma_xt(tb):
                    t0 = tb * 512
                    tiles = []
                    for ct in range(6):
                        xt = xtp.tile([128, 512], BF16, tag=f"xt{ct}",
                                      name=f"xt{ct}_{tb}")
                        nc.sync.dma_start(
                            xt, xT_d[ct * 128:(ct + 1) * 128, t0:t0 + 512])
                        tiles.append(xt)
                    return tiles

                def new_qtiles(tb):
                    q01 = qrp.tile([128, 512], F32R, tag="q01",
                                   name=f"q01_{tb}")
                    qv0 = qrp.tile([128, 512], F32R, tag="qv0",
                                   name=f"qv0_{tb}")
                    v2t = qrp.tile([64, 512], F32R, tag="v2t",
                                   name=f"v2t_{tb}")
                    return q01, qv0, v2t

                def proj_chunk(tb, xts, dests, mc):
                    mw = 128 if mc < 4 else 64
                    ps = projps.tile([mw, 512], F32, tag="proj",
                                     name=f"proj{tb}_{mc}")
                    for ct in range(6):
                        nc.tensor.matmul(
                            ps,
                            lhsT=wqkv_sb[ct][:, mc * 128:mc * 128 + mw],
                            rhs=xts[ct],
                            start=(ct == 0), stop=(ct == 5))
                    if mc in bias_sb:
                        nc.scalar.activation(dests[mc][:mw, :], ps,
                                             Act.Identity,
                                             bias=bias_sb[mc][:mw, :],
                                             scale=1.0)
                    else:
                        nc.vector.tensor_copy(dests[mc][:mw, :], ps)

                def vt_block(tb, qv0, v2t):
                    v_src = [qv0[64:128], kv1s[tb][64:128], v2t[0:64]]
                    v_idn = [ident[64:128, 64:128], ident[64:128, 64:128],
                             ident[0:64, 0:64]]
                    for h in range(3):
                        vt = projps.tile([128, 256], F32, tag="proj",
                                         name=f"vt{h}_{tb}")
                        for ck in range(4):
                            nc.tensor.transpose(
                                vt[:, ck * 64:(ck + 1) * 64],
                                v_src[h][:, ck * 128:(ck + 1) * 128]
                                .bitcast(F32),
                                v_idn[h])
                        for ck in range(4):
                            nc.vector.tensor_copy(
                                vss[h][tb][:, ck * 65:ck * 65 + 64],
                                vt[:, ck * 64:(ck + 1) * 64])

                def oproj_thunks(tb, attn):
                    # out-projection for q-block tb, cut into 8 filler
                    # chunks that interleave into the next block's
                    # attention stream (each: 3 matmuls + a PSUM copy).
                    thunks = []
                    for mt in range(4):
                        ys = ysp.tile([128, EMBED], BF16, tag="ys",
                                      name=f"ys{tb}_{mt}")
                        t_sl = slice(mt * 128, (mt + 1) * 128)
                        for n0, nw in ((0, 512), (512, 256)):
                            def th(tb=tb, attn=attn, ys=ys, t_sl=t_sl,
                                   n0=n0, nw=nw, mt=mt, last=(n0 == 512)):
                                yp = projps.tile([128, nw], F32,
                                                 tag="proj",
                                                 name=f"y{tb}_{mt}_{n0}")
                                for h in range(3):
                                    nc.tensor.matmul(
                                        yp,
                                        lhsT=attn[h][:, t_sl],
                                        rhs=wo_sb[h][:, n0:n0 + nw],
                                        start=(h == 0), stop=(h == 2))
                                nc.vector.tensor_copy(ys[:, n0:n0 + nw], yp)
                                if last:
                                    row0 = tb * 512 + mt * 128
                                    nc.sync.dma_start(
                                        y_d[row0:row0 + 128, :], ys)
                            thunks.append(th)
                    return thunks

                # prologue: block 0 projections + v transpose
                xts_cur = dma_xt(0)
                qcur = new_qtiles(0)
                dests0 = [qcur[0], k01s[0], qcur[1], kv1s[0], qcur[2]]
                for mc in range(5):
                    proj_chunk(0, xts_cur, dests0, mc)
                vt_block(0, qcur[1], qcur[2])

                oq = []   # deferred out-projection chunks, rationed so
                          # they fill the ACT-bound late blocks
                for tb in range(ntb):
                    t0 = tb * 512
                    q01, qv0, v2t = qcur
                    fillers = []
                    if tb + 1 < ntb:
                        xts_next = dma_xt(tb + 1)
                        qnext = new_qtiles(tb + 1)
                        destsn = [qnext[0], k01s[tb + 1], qnext[1],
                                  kv1s[tb + 1], qnext[2]]
                        # DVE-copied chunks first: their PSUM slots recycle
                        # fast, the two ACT-copied chunks go last.
                        for mc in (1, 3, 4, 0, 2):
                            fillers.append(
                                lambda mc=mc, xts=xts_next, ds=destsn:
                                proj_chunk(tb + 1, xts, ds, mc))

                    def q_ap(h):
                        return (q01[0:64], q01[64:128], qv0[0:64])[h]

                    def k_ap(h, kbi):
                        ktile = (k01s, k01s, kv1s)[h][kbi // 4]
                        base = 0 if h != 1 else 64
                        col = (kbi % 4) * 128
                        return ktile[base:base + 64, col:col + 128]

                    kbn = 4 * (tb + 1)
                    attn = [attnp.tile([64, 512], BF16, tag=f"attn{h}",
                                       name=f"attn{h}_{tb}")
                            for h in range(3)]
                    acc_sbs = []
                    recs = []
                    slot_ctr = [0]
                    for h in range(3):
                        acc = accps.tile([65, 512], F32, tag="acc",
                                         name=f"acc{tb}_{h}")

                        def pv(pt, g, h=h, acc=acc):
                            for j in range(2):
                                kbi = 2 * g + j
                                # diagonal blocks: columns u < kbi*128-t0
                                # are masked to zero, skip them (kbi == 0
                                # always covers the full width, so the
                                # start matmul initializes every column).
                                o = max(0, kbi * 128 - t0)
                                nc.tensor.matmul(
                                    acc[:, o:512],
                                    lhsT=vss[h][kbi // 4][
                                        :, (kbi % 4) * 65:(kbi % 4) * 65 + 65],
                                    rhs=pt[:, j * 512 + o:(j + 1) * 512],
                                    start=(kbi == 0), stop=(kbi == kbn - 1))

                        # S(g+1) then a filler chunk are emitted before
                        # PV(g): the in-order PE queue then has ~1.5us of
                        # independent matmuls between exp(g) and the PV
                        # that waits on it, hiding the ACT latency.
                        pend = None
                        for g in range(kbn // 2):
                            slot = slot_ctr[0]
                            slot_ctr[0] += 1
                            sp = spps.tile([128, 1024], F32, tag="s",
                                           name=f"s{tb}_{h}_{g}")
                            pt = ptp.tile([128, 1024], BF16, tag="p",
                                          name=f"p{tb}_{h}_{g}")
                            for j in range(2):
                                kbi = 2 * g + j
                                # skip fully-masked leading columns, but
                                # keep the moving dim >= 256 (fp32r runs
                                # at 1/4 rate below that)
                                oS = min(max(0, kbi * 128 - t0), 256)
                                nc.tensor.matmul(
                                    sp[:, j * 512 + oS:(j + 1) * 512],
                                    lhsT=k_ap(h, kbi),
                                    rhs=q_ap(h)[:, oS:],
                                    start=True, stop=True)
                            nc.scalar.activation(pt, sp, Act.Exp,
                                                 bias=0.0, scale=0.125)
                            for j in range(2):
                                kbi = 2 * g + j
                                if kbi >= 4 * tb:
                                    # diagonal block: causal predicate
                                    # keep where (t0+u) >= (kbi*128+si)
                                    nc.gpsimd.affine_select(
                                        out=pt[:, j * 512:(j + 1) * 512],
                                        in_=pt[:, j * 512:(j + 1) * 512],
                                        compare_op=mybir.AluOpType.is_ge,
                                        fill=0.0,
                                        base=t0 - kbi * 128,
                                        pattern=[[1, 512]],
                                        channel_multiplier=-1,
                                    )
                            if slot >= 2 and fillers:
                                fillers.pop(0)()
                            elif slot >= 28 and oq:
                                oq.pop(0)()
                            if pend is not None:
                                pv(*pend)
                            pend = (pt, g)
                        pv(*pend)
                        # stage the PSUM acc out now (PSUM has one DVE read
                        # port, so the normalize mul can have at most one
                        # PSUM operand); denominators on row 64.
                        acc_sb = recp.tile([65, 512], F32, tag="accsb",
                                           name=f"accsb{tb}_{h}")
                        nc.vector.tensor_copy(acc_sb, acc)
                        rec = recp.tile([65, 512], F32R, tag="rec",
                                        name=f"rec{tb}_{h}")
                        with nc.allow_low_precision(
                                reason="fp32r operand rounding"):
                            nc.vector.reciprocal(rec[64:65, :],
                                                 acc_sb[64:65, :])
                        acc_sbs.append(acc_sb)
                        recs.append(rec)
                    for th in fillers:
                        th()
                    if tb + 1 < ntb:
                        vt_block(tb + 1, qnext[1], qnext[2])
                        qcur = qnext
                    # normalize epilogues after all heads: the bc matmuls
                    # depend on the DVE reciprocal chain; emitting them here
                    # keeps them from blocking the S streams in the
                    # in-order PE queue.  Broadcast 1/denom across
                    # partitions 0:64 with a K=1 matmul whose operands both
                    # live at base partition 64.
                    for h in range(3):
                        bc = ypps.tile([64, 512], F32, tag="yp",
                                       name=f"bc{tb}_{h}")
                        nc.tensor.matmul(bc, lhsT=ones65[64:65, :],
                                         rhs=recs[h][64:65, :],
                                         start=True, stop=True)
                        nc.vector.tensor_mul(attn[h], acc_sbs[h][0:64, :],
                                             bc)
                    # out-projection is deferred: its chunks become
                    # rationed filler work inside later blocks' attention
                    # streams (consumed from slot 25, which only the
                    # ACT-bound late blocks reach).
                    oq.extend(oproj_thunks(tb, attn))
                while oq:
                    oq.pop(0)()
    nc.compile()
    return nc


_PROG_CACHE = {}


def _get_program(t=T):
    if t not in _PROG_CACHE:
        _PROG_CACHE[t] = build_program(t)
    return _PROG_CACHE[t]


def make_in_maps(x, wq, bq, wk, bk, wv, bv, wo):
    """Per-core inputs.  The k-bias is dropped entirely (softmax over s is
    invariant to terms constant in s, and (q+bq).(k+bk) - (q+bq).k is one),
    and the v-bias is folded into a constant output row bv @ wo.T that the
    host adds during unsharding (softmax rows sum to 1)."""
    zb = np.zeros(DH, dtype=np.float32)
    in_maps = []
    for core in range(NCORES):
        b = core // 4
        hs = (core % 4) * HPC
        sl = [slice((hs + h) * DH, (hs + h + 1) * DH) for h in range(HPC)]
        # columns: q0,q1 | k0,k1 | q2,v0 | k2,v1 | v2
        cols = [wq[sl[0]].T, wq[sl[1]].T, wk[sl[0]].T, wk[sl[1]].T,
                wq[sl[2]].T, wv[sl[0]].T, wk[sl[2]].T, wv[sl[1]].T,
                wv[sl[2]].T]
        biases = [bq[sl[0]], bq[sl[1]], zb, zb,
                  bq[sl[2]], zb, zb, zb, zb]
        wqkvT = np.ascontiguousarray(np.concatenate(cols, axis=1),
                                     dtype=ml_dtypes.bfloat16)
        bqkv = np.ascontiguousarray(
            np.concatenate(biases)[:, None], dtype=np.float32)
        ch = slice(hs * DH, (hs + HPC) * DH)
        woT = np.ascontiguousarray(
            wo[:, ch].T.astype(ml_dtypes.bfloat16))
        in_maps.append({
            "xT": np.ascontiguousarray(
                x[b].T.astype(ml_dtypes.bfloat16)),
            "wqkvT": wqkvT,
            "bqkv": bqkv,
            "woT": woT,
        })
    return in_maps


def run(inputs, t=T, trace=False, **kw):
    """Run on hardware; returns (y, BassKernelResults)."""
    arrs = {k: np.asarray(v, dtype=np.float32) for k, v in inputs.items()}
    nc = _get_program(t)
    in_maps = make_in_maps(**arrs)
    res = run_bass_kernel_spmd(nc, in_maps, list(range(NCORES)),
                               trace=trace, **kw)
    outs = [np.asarray(m["y"]).astype(np.float32) for m in res.results]
    # v-bias contribution: softmax rows sum to 1, so attn = P@v/D + bv and
    # the out-projection adds the constant row bv @ wo.T to every position.
    y_bias = (arrs["wo"] @ arrs["bv"]).astype(np.float32)
    y = np.empty((B, t, EMBED), dtype=np.float32)
    for b in range(B):
        y[b] = outs[4 * b] + outs[4 * b + 1] + outs[4 * b + 2] + outs[4 * b + 3]
        y[b] += y_bias
    return y, res


def kernel(**inputs):
    y, _ = run(inputs)
    return y
